# revision 31
# baseline (speedup 1.0000x reference)
"""BitLinear kernel for Trainium2, 8-core column-parallel.

Computes out = x @ (sign(W) * (weight_scale @ input_factor)).T
  x: [32, 8, 4096] f32, W: [11008, 4096] f32,
  weight_scale: [11008, 4] f32, input_factor: [4, 4096] f32
  -> out: [32, 8, 11008] f32

Sharding: column-parallel over out_features (11008 = 8 x 1376). Each core
gets its W / weight_scale row-shard plus replicated x / input_factor, and
produces out[:, core_slice]; host concatenates. No collectives.

Final dataflow (~73us vs the 159us v1 baseline, whose every MM ran
cold+isolated at (219+N)/1.2 ns because the ACT-Sign -> DVE-mul chain
starved the PE and HAM never warmed):
  - sign(W) is precomputed on HOST and shipped as fp8 e4m3 (+-1/0
    exact), pre-arranged chunk-major -- no DMA transpose, no ACT Sign
    pass, and the s stream is 1 byte/weight in HBM.
  - o-chunk OUTER loop (512/512/352), i-blocks in groups of 4.
  - value strips v[i,o] = f.T @ wsT are K=4 matmuls; 4 i-blocks run
    CONCURRENTLY via tile_position row-tiling (rows 32j..32j+3), one
    PSUM bank each.
  - wsg = s * v per group, engine-balanced against the PE period
    (~2.1us): blocks 2,3 are DVE tensor_muls straight from PSUM (1x
    mode is dtype-agnostic, so their s stays fp8 in SBUF); blocks 0,1
    are ACT copies PSUM->SBUF fp16 plus one batched DVE tensor_mul in
    2x mode, fed from the "F" half of s that the gpsimd SWDGE cast-DMA
    upcast to fp16 on the way in.
  - 8 main MMs per group (4 i-blocks x 2 token-blocks, N=Nc) issue
    back-to-back (~215 ns each) and accumulate out[t, o-chunk] in PSUM
    across all 32 i-blocks.
  - PSUM: 4 value banks + 2x2 double-buffered out banks = 8 exactly.
  - the NEXT (chunk, group)'s value MMs are issued before this group's
    main burst -- including across chunk boundaries -- so vps banks
    refill while mains run and the PE never idles long enough for HAM
    to re-throttle.
  - out evacuation of chunk c drips into the ACT slack of chunk c+1's
    groups 1-2; the last chunk evacuates tb0/tb1 on ACT/DVE in
    parallel to shorten the tail.
  - DMA: every piece is its own contiguous DRAM tensor, ordered by
    first use; fp16-bound pieces + xT on the gpsimd SWDGE ring, fp8
    pieces + out on the sync HWDGE ring. All triggers fire up front;
    Tile subtile deps gate the consumers.
"""

import sys

if "/opt/trn_rl_repo" not in sys.path:
    sys.path.insert(0, "/opt/trn_rl_repo")

import numpy as np

# ---------------------------------------------------------------------------
# problem constants (hardcoded per the self-contained-kernel contract)
B, S, IN, OUT, R = 32, 8, 4096, 11008, 4
T = B * S               # 256 tokens
NCORES = 8
OS = OUT // NCORES      # 1376 out-features per core
P = 128
N_IBLK = IN // P        # 32 i-blocks
NGRP = N_IBLK // 4      # 8 groups of 4 i-blocks
O_CHUNKS = [(0, 512), (512, 512), (1024, 352)]
# free-dim offsets of each chunk's region in the chunk-major s layout
S_OFF = [0, N_IBLK * 512, N_IBLK * 1024]
S_TOT = N_IBLK * OS     # 44032
# s is split by block pair: blocks 0,1 of each group ("F") are upcast to
# fp16 by the SWDGE cast-DMA and feed the DVE 2x-mode multiply; blocks
# 2,3 ("E") stay fp8 in SBUF and feed the 1x-from-PSUM multiplies, which
# are dtype-agnostic. Halves of the chunk-major layout, so offsets below
# are in "half" coordinates (2 blocks x Nc per group).
HALF_OFF = [o // 2 for o in S_OFF]
S_HTOT = S_TOT // 2     # 22016
# (start elem, length) pieces; one DRAM tensor per piece so every
# transfer is a single contiguous HBM read, ordered by first use
SH_PIECES = [
    (HALF_OFF[0], 2 * 1024), (HALF_OFF[0] + 2048, 2 * 1024),
    (HALF_OFF[0] + 4096, 2 * 1024), (HALF_OFF[0] + 6144, 2 * 1024),
    (HALF_OFF[1], 4 * 1024), (HALF_OFF[1] + 4096, 4 * 1024),
    (HALF_OFF[2], 4 * 704), (HALF_OFF[2] + 2816, 4 * 704),
]
XT_PIECES = [(0, 8 * T), (8 * T, 8 * T), (16 * T, 8 * T), (24 * T, 8 * T)]


def _install_tile_drain_patch():
    """This walrus build rejects >2 sync waits on one TPB_CTRL instruction;
    split the TileContext end-of-kernel drain into one drain per proc."""
    from concourse.tile import TileContext
    from concourse.vector_clock import ScopedClock
    from bass_rust import VectorClock

    if getattr(TileContext, "_drain_patch_installed", False):
        return

    def patched_drain_and_barrier(self, tick_clock, wait_clock):
        nc = self.nc
        gc = tick_clock.global_clock
        for i in range(27):
            v = gc[i]
            if v > 0:
                single = [0] * 27
                single[i] = v
                d = nc.sync.drain()
                wait_clock.add_sem_waits(
                    d.ins, ScopedClock({None: VectorClock(single)})
                )
        nc.all_engine_barrier()
        assert self.sems is not None
        popped = nc._tile_sem_poison_stack.pop()
        assert popped is self._sem_poison
        nc.clear_and_free_semaphores(list(self.sems.allocated().values()))
        nc.all_engine_barrier()

    TileContext._drain_and_barrier = patched_drain_and_barrier
    TileContext._drain_patch_installed = True


def _split_excess_waits(nc, max_waits=1):
    """This walrus build rejects instructions carrying more than ~2 sync
    waits. Move excess waits onto no-op instructions inserted immediately
    before the offender on the same engine (same semantics: the engine
    performs the same waits, in order, before executing the instruction)."""
    import concourse.mybir as mybir

    n_split = 0
    for fn in nc.m.functions:
        for bb in fn.blocks:
            insts = list(bb.instructions)
            new = []
            changed = False
            for inst in insts:
                si = inst.sync_info
                waits = list(si.on_wait) if si is not None else []
                if len(waits) > max_waits:
                    changed = True
                    n_split += 1
                    excess = waits[:-max_waits]
                    keep = waits[-max_waits:]
                    for i in range(0, len(excess), max_waits):
                        chunk = excess[i : i + max_waits]
                        nop = mybir.InstNoOp(
                            name=nc.get_next_instruction_name(),
                            sync_info=mybir.SyncInfo(
                                on_wait=chunk, on_update=[]
                            ),
                            bass_nofuse=True,
                            engine=inst.engine,
                        )
                        new.append(nop)
                    inst.sync_info = mybir.SyncInfo(
                        on_wait=keep, on_update=list(si.on_update)
                    )
                new.append(inst)
            if changed:
                bb.instructions = new
    return n_split


def build_nc():
    import concourse.bass as bass
    import concourse.mybir as mybir
    from concourse.bass import ts
    from concourse.tile import TileContext

    _install_tile_drain_patch()

    F32 = mybir.dt.float32
    F16 = mybir.dt.float16
    F8 = mybir.dt.float8e4
    nc = bass.Bass("TRN2", num_devices=NCORES)

    # host-prearranged inputs. s ships as fp8 e4m3 (+-1/0 exact); the
    # "F" half is upcast to fp16 by the SWDGE cast-DMA, the "E" half
    # stays fp8. Each piece is its own DRAM tensor so every DMA is one
    # fully contiguous HBM read.
    sf_exts = [
        nc.dram_tensor(f"sf{k}", [P, plen], F8, kind="ExternalInput").ap()
        for k, (lo, plen) in enumerate(SH_PIECES)
    ]
    se_exts = [
        nc.dram_tensor(f"se{k}", [P, plen], F8, kind="ExternalInput").ap()
        for k, (lo, plen) in enumerate(SH_PIECES)
    ]
    xT_exts = [
        nc.dram_tensor(f"xt{k}", [P, plen], F16, kind="ExternalInput").ap()
        for k, (lo, plen) in enumerate(XT_PIECES)
    ]
    frep_exts = [
        nc.dram_tensor("frep0", [P, P], F16, kind="ExternalInput").ap(),
        nc.dram_tensor(
            "frep1", [P, (NGRP - 1) * P], F16, kind="ExternalInput"
        ).ap(),
    ]
    wsrep_exts = [
        nc.dram_tensor("wsrep0", [P, 512], F16, kind="ExternalInput").ap(),
        nc.dram_tensor(
            "wsrep1", [P, OS - 512], F16, kind="ExternalInput"
        ).ap(),
    ]
    out_ext = nc.dram_tensor("out", [T, OS], F32, kind="ExternalOutput").ap()

    with TileContext(nc) as tc:
        with (
            tc.tile_pool(name="const", bufs=1) as cpool,
            tc.tile_pool(name="vsb", bufs=2) as vsbpool,
            tc.tile_pool(name="wsgp", bufs=2) as wsgpool,
            tc.tile_pool(name="outsb", bufs=2) as outsb,
            tc.tile_pool(name="vpsum", bufs=1, space="PSUM") as vpool,
            tc.tile_pool(name="opsum", bufs=2, space="PSUM") as opool,
        ):
            # resident SBUF inputs
            frep_sb = cpool.tile([P, NGRP * P], F16)
            wsrep_sb = cpool.tile([P, OS], F16)
            sF_sb = cpool.tile([P, S_HTOT], F16)
            sE_sb = cpool.tile([P, S_HTOT], F8)
            xT_sb = cpool.tile([P, N_IBLK * T], F16)

            # prefetch: tiny stationaries + xT on the sync HWDGE ring;
            # the fp8->fp16 cast-DMAs of s ride the gpsimd SWDGE queue so
            # the ACT engine stays free for value copies. Everything is
            # triggered up front; subtile deps gate the consumers.
            # group-0 / chunk-0 slices of the stationaries first so the
            # first value MMs unblock as early as possible
            nc.sync.dma_start(frep_sb[:, 0:P], frep_exts[0][:, :])
            nc.sync.dma_start(wsrep_sb[:, 0:512], wsrep_exts[0][:, :])
            # gpsimd (SWDGE) ring: fp16-target s pieces (cast) interleaved
            # with xT by first-use order (the plain xT traffic raises the
            # ring's throughput vs casts alone); sync ring: fp8 s pieces
            gring = [("sf", 0), ("xt", 0), ("sf", 1), ("xt", 1),
                     ("sf", 2), ("xt", 2), ("sf", 3), ("xt", 3),
                     ("sf", 4), ("sf", 5), ("sf", 6), ("sf", 7)]
            for kind, k in gring:
                if kind == "sf":
                    lo, plen = SH_PIECES[k]
                    nc.gpsimd.dma_start(
                        sF_sb[:, lo : lo + plen], sf_exts[k][:, :]
                    )
                else:
                    lo, plen = XT_PIECES[k]
                    nc.gpsimd.dma_start(
                        xT_sb[:, lo : lo + plen], xT_exts[k][:, :]
                    )
            for k, ((lo, plen), ext) in enumerate(zip(SH_PIECES, se_exts)):
                nc.sync.dma_start(sE_sb[:, lo : lo + plen], ext[:, :])
                if k == 0:
                    nc.sync.dma_start(
                        frep_sb[:, P:], frep_exts[1][:, :]
                    )
                    nc.sync.dma_start(
                        wsrep_sb[:, 512:], wsrep_exts[1][:, :]
                    )

            def emit_evac(ps, cc0, ccNc, tb, eng="scalar"):
                o_sb = outsb.tile(
                    [P, ccNc], F32, tag=f"osb{tb}", name=f"o_sb{tb}"
                )
                if eng == "scalar":
                    nc.scalar.copy(o_sb, ps)
                else:
                    nc.vector.tensor_copy(o_sb, ps)
                nc.sync.dma_start(out_ext[ts(tb, P), cc0 : cc0 + ccNc], o_sb)

            def issue_value(c, g):
                """4 K=4 value matmuls for (chunk c, group g), run
                concurrently in distinct 32-row groups of the PE array."""
                cc0, ccNc = O_CHUNKS[c]
                vtiles = [
                    vpool.tile([P, ccNc], F32, tag=f"v{j}", name=f"v{j}")
                    for j in range(4)
                ]
                for j in range(4):
                    nc.tensor.matmul(
                        vtiles[j],
                        frep_sb[32 * j : 32 * j + 4, g * P : (g + 1) * P],
                        wsrep_sb[32 * j : 32 * j + 4, cc0 : cc0 + ccNc],
                        start=True,
                        stop=True,
                        tile_position=(32 * j, 0),
                    )
                return vtiles

            pairs = [(c, g) for c in range(len(O_CHUNKS)) for g in range(NGRP)]
            pending_evac = None  # (out_ps, c0, Nc) of the previous chunk
            out_ps = None
            vps_next = issue_value(0, 0)
            for k, (c, g) in enumerate(pairs):
                c0, Nc = O_CHUNKS[c]
                if g == 0:
                    out_ps = [
                        opool.tile(
                            [P, Nc], F32, tag=f"out{tb}", name=f"out_ps{tb}"
                        )
                        for tb in range(2)
                    ]
                vps = vps_next

                # signed-weight build wsg = s * v, balanced so DVE and
                # ACT each finish inside one PE group-period:
                #  - blocks 2,3: DVE tensor_mul straight from PSUM (1x
                #    mode, dtype-agnostic -> s stays fp8)
                #  - blocks 0,1: ACT copies PSUM->SBUF fp16, then one
                #    batched DVE tensor_mul in 2x mode (fp16 SBUF)
                wsg = wsgpool.tile([P, 4 * Nc], F16, tag="wsg", name="wsg")
                lo2 = HALF_OFF[c] + g * 2 * Nc
                nc.vector.tensor_mul(
                    wsg[:, 2 * Nc : 3 * Nc], sE_sb[:, lo2 : lo2 + Nc], vps[2]
                )
                nc.vector.tensor_mul(
                    wsg[:, 3 * Nc : 4 * Nc],
                    sE_sb[:, lo2 + Nc : lo2 + 2 * Nc],
                    vps[3],
                )
                v_sb = vsbpool.tile([P, 2 * Nc], F16, tag="v_sb", name="v_sb")
                nc.scalar.copy(v_sb[:, 0 * Nc : 1 * Nc], vps[0])
                nc.scalar.copy(v_sb[:, 1 * Nc : 2 * Nc], vps[1])
                nc.vector.tensor_mul(
                    wsg[:, 0 : 2 * Nc], sF_sb[:, lo2 : lo2 + 2 * Nc], v_sb
                )

                # drip the PREVIOUS chunk's out evacuation into the ACT
                # slack of groups 1 and 2 (its PSUM is double-buffered)
                if pending_evac is not None and g in (1, 2):
                    pps, pc0, pNc = pending_evac
                    emit_evac(pps[g - 1], pc0, pNc, g - 1)
                    if g == 2:
                        pending_evac = None

                # issue the NEXT (chunk, group)'s value MMs before this
                # group's main burst so their PSUM banks refill while the
                # mains run -- including across chunk boundaries
                if k + 1 < len(pairs):
                    vps_next = issue_value(*pairs[k + 1])

                # main burst: 8 dense MMs accumulating out[t, o-chunk];
                # blocks 2,3 first -- their wsg slices land earliest
                last_pair = k == len(pairs) - 1
                if not last_pair:
                    for jj, j in enumerate((2, 3, 0, 1)):
                        ib = g * 4 + j
                        for tb in range(2):
                            nc.tensor.matmul(
                                out_ps[tb],
                                xT_sb[:, ib * T + tb * P : ib * T + tb * P + P],
                                wsg[:, j * Nc : (j + 1) * Nc],
                                start=(g == 0 and jj == 0),
                                stop=(g == NGRP - 1 and jj == 3),
                            )
                else:
                    # very last group: token-block-major so tb0's
                    # accumulation stops early and its evacuation (ACT)
                    # overlaps tb1's final matmuls; tb1 evacuates on DVE
                    for tb in range(2):
                        for jj, j in enumerate((2, 3, 0, 1)):
                            ib = g * 4 + j
                            nc.tensor.matmul(
                                out_ps[tb],
                                xT_sb[:, ib * T + tb * P : ib * T + tb * P + P],
                                wsg[:, j * Nc : (j + 1) * Nc],
                                start=False,
                                stop=(jj == 3),
                            )
                        emit_evac(
                            out_ps[tb], c0, Nc, tb,
                            eng="scalar" if tb == 0 else "vector",
                        )

                if g == NGRP - 1 and not last_pair:
                    # hand this chunk's evacuation to the next chunk's
                    # slack; the last chunk evacuated above
                    pending_evac = (out_ps, c0, Nc)

    _split_excess_waits(nc)
    return nc


_NC_CACHE = None


def make_in_maps(x, weight, weight_scale, input_factor):
    import ml_dtypes

    F8NP = ml_dtypes.float8_e4m3

    xf = np.ascontiguousarray(x.reshape(T, IN)).astype(np.float32)
    # xT_arr[p, ib*T + t] = x[t, ib*128 + p]
    xT_arr = (
        xf.T.reshape(N_IBLK, P, T).transpose(1, 0, 2).reshape(P, N_IBLK * T)
    ).astype(np.float16)
    xt_pieces = {
        f"xt{k}": np.ascontiguousarray(xT_arr[:, lo : lo + plen])
        for k, (lo, plen) in enumerate(XT_PIECES)
    }

    f32 = input_factor.astype(np.float32)
    in_maps = []
    for core in range(NCORES):
        sl = slice(core * OS, (core + 1) * OS)
        w_c = np.asarray(weight[sl], dtype=np.float32)      # [OS, IN]
        ws_c = np.asarray(weight_scale[sl], dtype=np.float32)  # [OS, R]

        # s chunk-major: s_arr[p, S_OFF[c] + ib*4? -> (ib within group):
        # region c holds [ib, Nc] blocks: s_arr[p, S_OFF[c] + ib*Nc + u]
        #   = sign(W[c0+u? no: = sign(w_c[o, i]) at i = ib*128+p, o = c0+u
        sT = np.sign(w_c).T.astype(F8NP)                    # [IN, OS]
        sT3 = sT.reshape(NGRP, 4, P, OS)                    # [g, j, p, o]
        fparts, eparts = [], []
        for (c0, Nc) in O_CHUNKS:
            # [g, j, p, Nc] -> [p, g, j, Nc], split j in {0,1} / {2,3}
            blk = sT3[:, :, :, c0 : c0 + Nc].transpose(2, 0, 1, 3)
            fparts.append(blk[:, :, 0:2, :].reshape(P, -1))
            eparts.append(blk[:, :, 2:4, :].reshape(P, -1))
        sf_arr = np.ascontiguousarray(np.concatenate(fparts, axis=1))
        se_arr = np.ascontiguousarray(np.concatenate(eparts, axis=1))
        s_pieces = {}
        for k, (lo, plen) in enumerate(SH_PIECES):
            s_pieces[f"sf{k}"] = np.ascontiguousarray(sf_arr[:, lo : lo + plen])
            s_pieces[f"se{k}"] = np.ascontiguousarray(se_arr[:, lo : lo + plen])

        # frep[32j + r, g*128 + ii] = f[r, (4g+j)*128 + ii]
        frep = np.zeros((P, NGRP * P), dtype=np.float16)
        for j in range(4):
            for r in range(R):
                for g in range(NGRP):
                    frep[32 * j + r, g * P : (g + 1) * P] = f32[
                        r, (4 * g + j) * P : (4 * g + j + 1) * P
                    ]
        # wsrep[32j + r, o] = ws_c[o, r]
        wsrep = np.zeros((P, OS), dtype=np.float16)
        for j in range(4):
            for r in range(R):
                wsrep[32 * j + r, :] = ws_c[:, r]

        in_maps.append(
            {
                **s_pieces,
                **xt_pieces,
                "frep0": np.ascontiguousarray(frep[:, :P]),
                "frep1": np.ascontiguousarray(frep[:, P:]),
                "wsrep0": np.ascontiguousarray(wsrep[:, :512]),
                "wsrep1": np.ascontiguousarray(wsrep[:, 512:]),
            }
        )
    return in_maps


def gather_out(results):
    outs = [results[c]["out"] for c in range(NCORES)]
    full = np.concatenate(outs, axis=1)  # [T, OUT]
    return np.ascontiguousarray(full.reshape(B, S, OUT).astype(np.float32))


def kernel(x, weight, weight_scale, input_factor):
    global _NC_CACHE
    from concourse.bass_utils import run_bass_kernel_spmd

    if _NC_CACHE is None:
        _NC_CACHE = build_nc()
    nc = _NC_CACHE

    in_maps = make_in_maps(x, weight, weight_scale, input_factor)
    res = run_bass_kernel_spmd(nc, in_maps, core_ids=list(range(NCORES)))
    return gather_out(res.results)


if __name__ == "__main__":
    # quick self-run with random data
    rng = np.random.default_rng(0)
    x = rng.standard_normal((B, S, IN), dtype=np.float32)
    w = rng.standard_normal((OUT, IN), dtype=np.float32)
    ws = rng.standard_normal((OUT, R), dtype=np.float32)
    f = rng.standard_normal((R, IN), dtype=np.float32)
    out = kernel(x=x, weight=w, weight_scale=ws, input_factor=f)
    wv = ws @ f
    expected = np.einsum("bsi,oi->bso", x, np.sign(w) * wv)
    rel = np.abs(out - expected).max() / np.abs(expected).max()
    print("rel err:", rel)
